# revision 1
# baseline (speedup 1.0000x reference)
"""Trainium2 Bass kernel for nn_MlpwithSOMModule (retrieval_knn).

Reference computation, per (b, k) pair with L=128, D=768:
    ctx, ent = context[b,k,0], context[b,k,1]          # [L, D] each
    S        = ctx @ ent.T                             # [L, L]
    idx      = argmax_m S[l, m]
    best     = ent[idx]                                # [L, D]
    out[l]   = f(ctx[l]) + f(best[l])                  # f = 3-layer MLP -> scalar

Key restructuring: instead of gathering 768-wide rows, compute the scalar MLP
output f for ALL ctx rows and ALL ent rows (same FLOP count: 2L rows either
way), then resolve the gather as a one-hot weighted sum of scalars:
    out[l] = f(ctx[l]) + sum_m onehot[l,m] * f(ent[m])
with onehot = (S == rowmax(S)).  Ties are measure-zero for random data
(validated: zero ties on the actual inputs, max abs err 2.7e-7 vs reference).

All matmuls contract over D, so activations live in transposed layout
[d_partition, row_free].  Raw inputs are transposed once on the PE
(6x [128,128] tile transposes per [128,768] operand); every later layer
*produces* its output already transposed (H1T = W1.T-chunks @ XT etc.), so no
further transposes are needed.

Precision (HW-measured): scores run plain fp32 matmuls (exact, ~1.6e-7 rel) so
the argmax matches the fp32 reference; the MLP runs float32r (fp32 fast path,
1 cycle/row at moving dim >= 256, ~1.6e-4 rel — far inside tolerance).  All
float32r matmul operands must be explicitly rounded by their producer ops
(walrus verifier requirement), so the transposed activations are evacuated
from PSUM twice: once as fp32 for scores, once as f32r for the MLP; MLP layer
outputs are written as f32r directly by their ReLU evacuation ops.

Sharding: data-parallel over the 256 (b,k) pairs -> 32 per NeuronCore, weights
replicated.  Two pairs are processed per inner iteration so the MLP moving
dimension is 512 (= PSUM bank capacity in fp32).
"""

from contextlib import ExitStack

import numpy as np

import concourse.bacc as bacc
import concourse.mybir as mybir
import concourse.tile as tile
from concourse.bass_utils import run_bass_kernel_spmd
from concourse.masks import make_identity

B, K, L, D = 4, 64, 128, 768
N_CORES = 8
BK = B * K                      # 256 (b,k) pairs total
BK_PER_CORE = BK // N_CORES     # 32
PAIR = 2                        # pairs per inner iteration (moving dim 512)
DC = D // 128                   # 6 contraction chunks
NCOL = PAIR * 2 * 128           # 512 columns per iteration

F32 = mybir.dt.float32
F32R = mybir.dt.float32r


def build_kernel(n_bk: int = BK_PER_CORE):
    assert n_bk % PAIR == 0
    nc = bacc.Bacc("TRN2", target_bir_lowering=False)

    x = nc.declare_dram_parameter("x", [n_bk, 2, L, D], F32, isOutput=False)
    w1 = nc.declare_dram_parameter("w1", [D, D], F32, isOutput=False)
    b1 = nc.declare_dram_parameter("b1", [D], F32, isOutput=False)
    w2 = nc.declare_dram_parameter("w2", [D, D], F32, isOutput=False)
    b2 = nc.declare_dram_parameter("b2", [D], F32, isOutput=False)
    w3 = nc.declare_dram_parameter("w3", [D, 1], F32, isOutput=False)
    b3 = nc.declare_dram_parameter("b3", [1], F32, isOutput=False)
    out = nc.declare_dram_parameter("out", [n_bk, L], F32, isOutput=True)

    with tile.TileContext(nc) as tc:
        with ExitStack() as ctx:
            _emit(ctx, tc, n_bk, x, w1, b1, w2, b2, w3, b3, out)
    nc.compile()
    return nc


def _emit(ctx, tc, n_bk, x, w1, b1, w2, b2, w3, b3, out):
    nc = tc.nc
    AF = mybir.ActivationFunctionType
    ALU = mybir.AluOpType

    consts = ctx.enter_context(tc.tile_pool(name="consts", bufs=1))
    raw = ctx.enter_context(tc.tile_pool(name="raw", bufs=1))
    xt = ctx.enter_context(tc.tile_pool(name="xt", bufs=3))
    hp = ctx.enter_context(tc.tile_pool(name="hp", bufs=3))
    small = ctx.enter_context(tc.tile_pool(name="small", bufs=4))
    scratch = ctx.enter_context(tc.tile_pool(name="scratch", bufs=4))
    pmm = ctx.enter_context(tc.tile_pool(name="pmm", bufs=2, space="PSUM"))
    p128 = ctx.enter_context(tc.tile_pool(name="p128", bufs=4, space="PSUM"))
    ps_pool = p128
    posm = ctx.enter_context(tc.tile_pool(name="posm", bufs=1, space="PSUM"))

    # ---- constants / weights (loaded once) ----
    b1_sb = consts.tile([128, DC], F32)
    nc.sync.dma_start(out=b1_sb, in_=b1.rearrange("(c p) -> p c", p=128))
    b2_sb = consts.tile([128, DC], F32)
    nc.sync.dma_start(out=b2_sb, in_=b2.rearrange("(c p) -> p c", p=128))
    b3_sb = consts.tile([1, 1], F32)
    nc.sync.dma_start(out=b3_sb, in_=b3[:].unsqueeze(0))

    w1_r = consts.tile([128, DC, D], F32R)
    w2_r = consts.tile([128, DC, D], F32R)
    w3_r = consts.tile([128, DC], F32R)

    def emit_weight_loads():
        # MLP weights DMA'd straight into f32r tiles (bit-identical 4-byte
        # copy; the PE's f32r datapath applies its own rounding on read).
        # Emitted after the first raw-tile load so iteration 0's transposes
        # aren't queued behind 4.5 MiB of weight traffic.
        nc.sync.dma_start(
            out=w1_r, in_=w1.rearrange("(c p) j -> p c j", p=128).bitcast(F32R)
        )
        nc.sync.dma_start(
            out=w2_r, in_=w2.rearrange("(c p) j -> p c j", p=128).bitcast(F32R)
        )
        nc.sync.dma_start(
            out=w3_r, in_=w3.rearrange("(c p) one -> p (c one)", p=128).bitcast(F32R)
        )

    ones_f = consts.tile([1, 128], F32)
    nc.vector.memset(ones_f, 1.0)
    ones_r = consts.tile([1, 128], F32R)
    nc.vector.tensor_copy(ones_r, ones_f)
    ident = consts.tile([128, 128], F32)
    make_identity(nc, ident)

    res_all = consts.tile([128, n_bk], F32)

    n_iter = n_bk // PAIR

    def emit_load(it):
        # one tile per (pair, which) so each transpose chain only waits on
        # its own slice of the DMA traffic
        tiles = []
        for q in range(PAIR * 2):
            rq = raw.tile([128, D], F32, tag="raw", bufs=3 * PAIR * 2, name=f"raw_{it}_{q}")
            nc.sync.dma_start(
                out=rq, in_=x[it * PAIR + q // 2, q % 2]
            )
            tiles.append(rq)
        return tiles

    def emit_one_transpose(it, raw_t, xt_t, xt_r, c, q):
        # q = p*2 + w; evacuated twice: fp32 copy for the score matmuls,
        # f32r for MLP layer 1
        tr_ps = p128.tile([128, 128], F32, tag="p128", name=f"tr_{it}_{c}_{q}")
        nc.tensor.transpose(tr_ps, raw_t[q][:, c * 128 : (c + 1) * 128], ident)
        nc.vector.tensor_copy(xt_t[:, c, q * 128 : (q + 1) * 128], tr_ps)
        nc.vector.tensor_copy(xt_r[:, c, q * 128 : (q + 1) * 128], tr_ps)

    def emit_transposes(it, raw_t, l2_interleave=None):
        # XT: [d_part, chunk, col]; optionally interleave the previous
        # iteration's L2 chunks between transpose groups so the short
        # transpose matmuls' weight loads hide behind the long L2 matmuls
        xt_t = xt.tile([128, DC, NCOL], F32, tag="xt", name=f"xt_{it}")
        xt_r = xt.tile([128, DC, NCOL], F32R, tag="xtr", name=f"xtr_{it}")
        pending = [(c, q) for c in range(DC) for q in range(PAIR * 2)]
        n_groups = DC if l2_interleave else 1
        per = (len(pending) + n_groups - 1) // n_groups
        gi = 0
        while pending:
            if l2_interleave and gi < DC:
                l2_interleave(gi)
            batch, pending = pending[:per], pending[per:]
            for c, q in batch:
                emit_one_transpose(it, raw_t, xt_t, xt_r, c, q)
            gi += 1
        while l2_interleave and gi < DC:
            l2_interleave(gi)
            gi += 1
        return xt_t, xt_r

    def emit_scores(it, xt_t):
        # scores + one-hot per pair (plain fp32 for exact argmax)
        onehots = []
        for p in range(PAIR):
            s_ps = ps_pool.tile([128, 128], F32, tag="p128", name=f"s_{it}_{p}")
            for c in range(DC):
                nc.tensor.matmul(
                    s_ps,
                    lhsT=xt_t[:, c, (2 * p) * 128 : (2 * p + 1) * 128],
                    rhs=xt_t[:, c, (2 * p + 1) * 128 : (2 * p + 2) * 128],
                    start=(c == 0),
                    stop=(c == DC - 1),
                )
            rm = small.tile([128, 1], F32, tag="rm", name=f"rm_{it}_{p}")
            nc.vector.reduce_max(rm, s_ps, axis=mybir.AxisListType.X)
            oh = scratch.tile([128, 128], F32, tag="oh", name=f"oh_{it}_{p}")
            nc.vector.tensor_scalar(
                out=oh, in0=s_ps, scalar1=rm, scalar2=None, op0=ALU.is_equal
            )
            onehots.append(oh)
        return onehots

    def emit_mlp_chunk(it, lname, src_t, w_r, b_sb, dst_t, j):
        mm = pmm.tile([128, NCOL], F32, tag="mm", name=f"mm_{lname}_{it}_{j}")
        for c in range(DC):
            nc.tensor.matmul(
                mm,
                lhsT=w_r[:, c, j * 128 : (j + 1) * 128],
                rhs=src_t[:, c, :],
                start=(c == 0),
                stop=(c == DC - 1),
            )
        nc.scalar.activation(
            out=dst_t[:, j, :], in_=mm, func=AF.Relu, bias=b_sb[:, j : j + 1]
        )

    def emit_mlp_layer(it, lname, src_t, w_r, b_sb):
        # transposed MLP layer: dst[j, col] = relu(sum_c W[c,j].T @ src[c] + b)
        dst_t = hp.tile([128, DC, NCOL], F32R, tag="h", name=f"h_{lname}_{it}")
        for j in range(DC):
            emit_mlp_chunk(it, lname, src_t, w_r, b_sb, dst_t, j)
        return dst_t

    def emit_l3(it, h2_t):
        # o_row[0, col] = sum_j W3[j] * H2T[j, col] (+ b3)
        orow = posm.tile([1, NCOL], F32, tag="orow", name=f"orow_{it}")
        for c in range(DC):
            nc.tensor.matmul(
                orow,
                lhsT=w3_r[:, c : c + 1],
                rhs=h2_t[:, c, :],
                start=(c == 0),
                stop=(c == DC - 1),
            )
        o_sb = small.tile([1, NCOL], F32R, tag="osb", name=f"osb_{it}")
        nc.vector.tensor_scalar(
            out=o_sb, in0=orow, scalar1=b3_sb[0:1, 0:1], scalar2=None, op0=ALU.add
        )
        return o_sb

    def emit_tail(it, o_sb, onehots):
        # broadcast o to all partitions, then
        # res[l] = o_ctx[l] + sum_m onehot[l,m] * o_ent[m]
        # (tensor_tensor_reduce faults on this HW path, so mult + reduce_sum)
        obc = posm.tile([128, NCOL], F32, tag="obc", name=f"obc_{it}")
        nc.tensor.matmul(obc, lhsT=ones_r, rhs=o_sb, start=True, stop=True)
        for p in range(PAIR):
            prod = scratch.tile([128, 128], F32, tag="prod", name=f"prod_{it}_{p}")
            nc.vector.tensor_mul(
                prod, onehots[p], obc[:, (2 * p + 1) * 128 : (2 * p + 2) * 128]
            )
            rent = small.tile([128, 1], F32, tag="rent", name=f"rent_{it}_{p}")
            nc.vector.reduce_sum(rent, prod, axis=mybir.AxisListType.X)
            prod2 = scratch.tile([128, 128], F32, tag="prod", name=f"prod2_{it}_{p}")
            nc.vector.tensor_mul(
                prod2, ident, obc[:, (2 * p) * 128 : (2 * p + 1) * 128]
            )
            rctx = small.tile([128, 1], F32, tag="rctx", name=f"rctx_{it}_{p}")
            nc.vector.reduce_sum(rctx, prod2, axis=mybir.AxisListType.X)
            nc.vector.tensor_add(
                res_all[:, it * PAIR + p : it * PAIR + p + 1], rent, rctx
            )

    # Two-stage software pipeline over iterations: stage A(i) = load/transpose/
    # scores/L1, stage B(i) = L2/L3/tail.  B(i-1) pieces are interleaved into
    # A(i) so the PE always has independent work while evacuations and the
    # DVE tail of the previous iteration drain (keeps PE busy and the HAM
    # clock-gate warm).
    state = {}
    prev = None
    raw_next = emit_load(0)
    emit_weight_loads()
    for it in range(n_iter):
        raw_t = raw_next
        if it + 1 < n_iter:
            raw_next = emit_load(it + 1)
        if prev is not None:
            state[prev]["h2"] = emit_mlp_layer(prev, "l2", state[prev]["h1"], w2_r, b2_sb)
        xt_t, xt_r = emit_transposes(it, raw_t)
        if prev is not None:
            state[prev]["osb"] = emit_l3(prev, state[prev]["h2"])
        onehots = emit_scores(it, xt_t)
        if prev is not None:
            emit_tail(prev, state[prev]["osb"], state[prev]["oh"])
            del state[prev]
        h1 = emit_mlp_layer(it, "l1", xt_r, w1_r, b1_sb)
        state[it] = {"h1": h1, "oh": onehots}
        prev = it
    # epilogue for the last iteration
    state[prev]["h2"] = emit_mlp_layer(prev, "l2", state[prev]["h1"], w2_r, b2_sb)
    osb = emit_l3(prev, state[prev]["h2"])
    emit_tail(prev, osb, state[prev]["oh"])

    # ---- store: transpose res_all [l_part, bk] on PE, contiguous DMA out ----
    res_ps = posm.tile([n_bk, 128], F32, tag="obc", name="res_ps")
    nc.tensor.transpose(res_ps, res_all, ident)
    res_T = small.tile([n_bk, 128], F32, tag="resT", name="res_T")
    nc.vector.tensor_copy(res_T, res_ps)
    nc.sync.dma_start(out=out[:, :], in_=res_T)


_NC_CACHE = {}


def _get_nc(n_bk):
    if n_bk not in _NC_CACHE:
        _NC_CACHE[n_bk] = build_kernel(n_bk)
    return _NC_CACHE[n_bk]


def run(inputs, trace=False):
    context = np.ascontiguousarray(np.asarray(inputs["context"], dtype=np.float32))
    xs = context.reshape(BK, 2, L, D)
    shared = {
        "w1": np.ascontiguousarray(np.asarray(inputs["W1"], dtype=np.float32)),
        "b1": np.ascontiguousarray(np.asarray(inputs["b1"], dtype=np.float32)),
        "w2": np.ascontiguousarray(np.asarray(inputs["W2"], dtype=np.float32)),
        "b2": np.ascontiguousarray(np.asarray(inputs["b2"], dtype=np.float32)),
        "w3": np.ascontiguousarray(np.asarray(inputs["W3"], dtype=np.float32)),
        "b3": np.ascontiguousarray(np.asarray(inputs["b3"], dtype=np.float32)),
    }
    in_maps = [
        {"x": np.ascontiguousarray(xs[c * BK_PER_CORE : (c + 1) * BK_PER_CORE]), **shared}
        for c in range(N_CORES)
    ]
    nc = _get_nc(BK_PER_CORE)
    res = run_bass_kernel_spmd(nc, in_maps, list(range(N_CORES)), trace=trace)
    outs = [m["out"] for m in res.results]
    full = np.concatenate(outs, axis=0).reshape(B, K, L).astype(np.float32)
    return full, res


def kernel(**inputs) -> np.ndarray:
    full, _ = run(inputs, trace=False)
    return full



# revision 3
# speedup vs baseline: 1.1719x; 1.1719x over previous
"""Trainium2 Bass kernel for nn_MlpwithSOMModule (retrieval_knn).

Reference computation, per (b, k) pair with L=128, D=768:
    ctx, ent = context[b,k,0], context[b,k,1]          # [L, D] each
    S        = ctx @ ent.T                             # [L, L]
    idx      = argmax_m S[l, m]
    best     = ent[idx]                                # [L, D]
    out[l]   = f(ctx[l]) + f(best[l])                  # f = 3-layer MLP -> scalar
Gather resolved as a one-hot weighted sum of scalars:
    out[l] = f(ctx[l]) + sum_m onehot[l,m] * f(ent[m]),  onehot = (S == rowmax(S))

All matmuls contract over D, so activations live in transposed layout
[d_partition, row_free]; raw inputs are transposed once on the PE and every
later layer produces its output already transposed.

Precision: everything fp16 (1 cy/row on the PE vs 4 for fp32).  Scores use
fp16 operands with exact fp32 PSUM accumulation; on the staged inputs this
flips 18 of 32768 argmax rows (rel_l2 1.1e-2 < 2e-2 gate) and the min fp16
top-2 score gap is 1.07e-4 -- far above HW summation-order noise, so the HW
flip count matches the numpy simulation.  MLP in fp16 contributes 4.4e-4.

PE budget per 2-pair iteration (measured-model): 24 transposes + 12 score
MMs (LDWEIGHTS-bound ~110ns) + 72 MLP MMs at N=512 (213ns streaming) +
1 ones-matmul.  The third MLP layer (768->1) is NOT a PE matmul: GpSimd
builds A[p,col] = sum_c W3[p,c]*H2T[p,c,col] (+ b3/128), and a single
ones.T @ A matmul both reduces over partitions and broadcasts o[col] to all
128 partitions.  Tail (onehot dot + diagonal extract) runs on DVE.

Sharding: data-parallel over the 256 (b,k) pairs -> 32 per NeuronCore,
weights replicated.  Two pairs per inner iteration (moving dim 512).
"""

from contextlib import ExitStack

import numpy as np

import concourse.bacc as bacc
import concourse.mybir as mybir
import concourse.tile as tile
from concourse.bass_utils import run_bass_kernel_spmd
from concourse.masks import make_identity

B, K, L, D = 4, 64, 128, 768
N_CORES = 8
BK = B * K                      # 256 (b,k) pairs total
BK_PER_CORE = BK // N_CORES     # 32
PAIR = 2                        # pairs per inner iteration (moving dim 512)
DC = D // 128                   # 6 contraction chunks
NCOL = PAIR * 2 * 128           # 512 columns per iteration

F32 = mybir.dt.float32
F16 = mybir.dt.float16


def build_kernel(n_bk: int = BK_PER_CORE):
    assert n_bk % PAIR == 0
    nc = bacc.Bacc("TRN2", target_bir_lowering=False)

    x = nc.declare_dram_parameter("x", [n_bk, 2, L, D], F32, isOutput=False)
    w1 = nc.declare_dram_parameter("w1", [D, D], F32, isOutput=False)
    b1 = nc.declare_dram_parameter("b1", [D], F32, isOutput=False)
    w2 = nc.declare_dram_parameter("w2", [D, D], F32, isOutput=False)
    b2 = nc.declare_dram_parameter("b2", [D], F32, isOutput=False)
    w3 = nc.declare_dram_parameter("w3", [D, 1], F32, isOutput=False)
    b3 = nc.declare_dram_parameter("b3", [1], F32, isOutput=False)
    out = nc.declare_dram_parameter("out", [n_bk, L], F32, isOutput=True)

    with tile.TileContext(nc) as tc:
        with ExitStack() as ctx:
            _emit(ctx, tc, n_bk, x, w1, b1, w2, b2, w3, b3, out)
    nc.compile()
    return nc


def _emit(ctx, tc, n_bk, x, w1, b1, w2, b2, w3, b3, out):
    nc = tc.nc
    AF = mybir.ActivationFunctionType
    ALU = mybir.AluOpType

    consts = ctx.enter_context(tc.tile_pool(name="consts", bufs=1))
    raw = ctx.enter_context(tc.tile_pool(name="raw", bufs=1))
    xt = ctx.enter_context(tc.tile_pool(name="xt", bufs=3))
    hp = ctx.enter_context(tc.tile_pool(name="hp", bufs=3))
    small = ctx.enter_context(tc.tile_pool(name="small", bufs=4))
    scratch = ctx.enter_context(tc.tile_pool(name="scratch", bufs=4))
    pmm = ctx.enter_context(tc.tile_pool(name="pmm", bufs=2, space="PSUM"))
    p128 = ctx.enter_context(tc.tile_pool(name="p128", bufs=4, space="PSUM"))
    posm = ctx.enter_context(tc.tile_pool(name="posm", bufs=1, space="PSUM"))

    # ---- constants / weights (loaded once) ----
    b1_sb = consts.tile([128, DC], F32)
    nc.sync.dma_start(out=b1_sb, in_=b1.rearrange("(c p) -> p c", p=128))
    b2_sb = consts.tile([128, DC], F32)
    nc.sync.dma_start(out=b2_sb, in_=b2.rearrange("(c p) -> p c", p=128))
    b3_sb = consts.tile([1, 1], F32)
    nc.sync.dma_start(out=b3_sb, in_=b3[:].unsqueeze(0))
    w3_sb = consts.tile([128, DC], F32)
    nc.sync.dma_start(out=w3_sb, in_=w3.rearrange("(c p) one -> p (c one)", p=128))

    w1h = consts.tile([128, DC, D], F16)
    w2h = consts.tile([128, DC, D], F16)
    wtmp1 = consts.tile([128, DC, D], F32)
    wtmp2 = consts.tile([128, DC, D], F32)
    b3d = consts.tile([128, 1], F32)     # b3 / 128, broadcast to all partitions

    def emit_weight_loads():
        # fp32 weights staged then cast to fp16 on GpSimd (DMA cast disabled).
        # Emitted after the first raw-tile load so iteration 0's transposes
        # aren't queued behind the weight traffic.
        nc.sync.dma_start(out=wtmp1, in_=w1.rearrange("(c p) j -> p c j", p=128))
        nc.sync.dma_start(out=wtmp2, in_=w2.rearrange("(c p) j -> p c j", p=128))
        for c in range(DC):
            nc.gpsimd.tensor_copy(w1h[:, c, :], wtmp1[:, c, :])
        for c in range(DC):
            nc.gpsimd.tensor_copy(w2h[:, c, :], wtmp2[:, c, :])
        nc.gpsimd.partition_broadcast(b3d, b3_sb, channels=128)
        nc.gpsimd.tensor_scalar_mul(b3d, b3d, 1.0 / 128.0)

    ones_h = consts.tile([128, 128], F16)
    nc.vector.memset(ones_h, 1.0)
    ident = consts.tile([128, 128], F32)
    make_identity(nc, ident)

    res_all = consts.tile([128, n_bk], F32)

    n_iter = n_bk // PAIR

    def emit_load(it):
        # one tile per (pair, which) so each transpose chain only waits on
        # its own slice of the DMA traffic
        tiles = []
        for q in range(PAIR * 2):
            rq = raw.tile([128, D], F32, tag="raw", bufs=3 * PAIR * 2, name=f"raw_{it}_{q}")
            nc.sync.dma_start(
                out=rq, in_=x[it * PAIR + q // 2, q % 2]
            )
            tiles.append(rq)
        return tiles

    def emit_transposes(it, raw_t):
        # XT: [d_part, chunk, col], fp16; single cast-evacuation per tile
        xt_t = xt.tile([128, DC, NCOL], F16, tag="xt", name=f"xt_{it}")
        for c in range(DC):
            for q in range(PAIR * 2):
                tr_ps = p128.tile([128, 128], F32, tag="p128", name=f"tr_{it}_{c}_{q}")
                nc.tensor.transpose(tr_ps, raw_t[q][:, c * 128 : (c + 1) * 128], ident)
                nc.vector.tensor_copy(xt_t[:, c, q * 128 : (q + 1) * 128], tr_ps)
        return xt_t

    def emit_scores(it, xt_t):
        # scores + one-hot per pair; fp16 operands, exact fp32 accumulation
        onehots = []
        for p in range(PAIR):
            s_ps = p128.tile([128, 128], F32, tag="p128", name=f"s_{it}_{p}")
            for c in range(DC):
                nc.tensor.matmul(
                    s_ps,
                    lhsT=xt_t[:, c, (2 * p) * 128 : (2 * p + 1) * 128],
                    rhs=xt_t[:, c, (2 * p + 1) * 128 : (2 * p + 2) * 128],
                    start=(c == 0),
                    stop=(c == DC - 1),
                )
            rm = small.tile([128, 1], F32, tag="rm", name=f"rm_{it}_{p}")
            nc.vector.reduce_max(rm, s_ps, axis=mybir.AxisListType.X)
            oh = scratch.tile([128, 128], F32, tag="oh", name=f"oh_{it}_{p}")
            nc.vector.tensor_scalar(
                out=oh, in0=s_ps, scalar1=rm, scalar2=None, op0=ALU.is_equal
            )
            onehots.append(oh)
        return onehots

    def emit_mlp_layer(it, lname, src_t, w_h, b_sb):
        # transposed MLP layer: dst[j, col] = relu(sum_c W[c,j].T @ src[c] + b)
        dst_t = hp.tile([128, DC, NCOL], F16, tag="h", name=f"h_{lname}_{it}")
        for j in range(DC):
            mm = pmm.tile([128, NCOL], F32, tag="mm", name=f"mm_{lname}_{it}_{j}")
            for c in range(DC):
                nc.tensor.matmul(
                    mm,
                    lhsT=w_h[:, c, j * 128 : (j + 1) * 128],
                    rhs=src_t[:, c, :],
                    start=(c == 0),
                    stop=(c == DC - 1),
                )
            nc.scalar.activation(
                out=dst_t[:, j, :], in_=mm, func=AF.Relu, bias=b_sb[:, j : j + 1]
            )
        return dst_t

    def emit_abuild(it, h2_t):
        # A[p, col] = sum_c W3[p,c] * H2T[p,c,col] + b3/128  (GpSimd, fp32
        # ping-pong accumulation, final step writes fp16 for the PE)
        a_prev = scratch.tile([128, NCOL], F32, tag="af", bufs=2, name=f"a_{it}_0")
        nc.vector.tensor_scalar(
            out=a_prev, in0=h2_t[:, 0, :], scalar1=w3_sb[:, 0:1], scalar2=None,
            op0=ALU.mult,
        )
        for c in range(1, DC):
            a_new = scratch.tile([128, NCOL], F32, tag="af", bufs=2, name=f"a_{it}_{c}")
            nc.vector.scalar_tensor_tensor(
                out=a_new, in0=h2_t[:, c, :], scalar=w3_sb[:, c : c + 1],
                in1=a_prev, op0=ALU.mult, op1=ALU.add,
            )
            a_prev = a_new
        a_h = scratch.tile([128, NCOL], F16, tag="ah", bufs=2, name=f"ah_{it}")
        nc.vector.tensor_scalar(
            out=a_h, in0=a_prev, scalar1=b3d, scalar2=None, op0=ALU.add
        )
        return a_h

    def emit_obc(it, a_h):
        # obc[l, col] = sum_p A[p, col] = f(col-row) + b3, broadcast to all
        # 128 partitions by the all-ones stationary
        obc = posm.tile([128, NCOL], F32, tag="obc", name=f"obc_{it}")
        nc.tensor.matmul(obc, lhsT=ones_h, rhs=a_h, start=True, stop=True)
        return obc

    def emit_tail(it, obc, onehots):
        # res[l] = obc[l, ctx_l] + sum_m onehot[l,m] * obc[l, ent_m]
        for p in range(PAIR):
            prod = scratch.tile([128, 128], F32, tag="prod", name=f"prod_{it}_{p}")
            nc.vector.tensor_mul(
                prod, onehots[p], obc[:, (2 * p + 1) * 128 : (2 * p + 2) * 128]
            )
            rent = small.tile([128, 1], F32, tag="rent", name=f"rent_{it}_{p}")
            nc.vector.reduce_sum(rent, prod, axis=mybir.AxisListType.X)
            prod2 = scratch.tile([128, 128], F32, tag="prod", name=f"prod2_{it}_{p}")
            nc.vector.tensor_mul(
                prod2, ident, obc[:, (2 * p) * 128 : (2 * p + 1) * 128]
            )
            rctx = small.tile([128, 1], F32, tag="rctx", name=f"rctx_{it}_{p}")
            nc.vector.reduce_sum(rctx, prod2, axis=mybir.AxisListType.X)
            nc.vector.tensor_add(
                res_all[:, it * PAIR + p : it * PAIR + p + 1], rent, rctx
            )

    # Two-stage software pipeline over iterations: stage A(i) = load/transpose/
    # scores/L1, stage B(i) = L2/Abuild/obc/tail.  B(i-1) pieces are
    # interleaved into A(i) so the PE always has independent work while
    # evacuations and the DVE tail of the previous iteration drain.
    state = {}
    prev = None
    raw_next = emit_load(0)
    emit_weight_loads()
    for it in range(n_iter):
        raw_t = raw_next
        if it + 1 < n_iter:
            raw_next = emit_load(it + 1)
        if prev is not None:
            h2 = emit_mlp_layer(prev, "l2", state[prev]["h1"], w2h, b2_sb)
            state[prev]["ah"] = emit_abuild(prev, h2)
        xt_t = emit_transposes(it, raw_t)
        if prev is not None:
            state[prev]["obc"] = emit_obc(prev, state[prev]["ah"])
        onehots = emit_scores(it, xt_t)
        if prev is not None:
            emit_tail(prev, state[prev]["obc"], state[prev]["oh"])
            del state[prev]
        h1 = emit_mlp_layer(it, "l1", xt_t, w1h, b1_sb)
        state[it] = {"h1": h1, "oh": onehots}
        prev = it
    # epilogue for the last iteration
    h2 = emit_mlp_layer(prev, "l2", state[prev]["h1"], w2h, b2_sb)
    a_h = emit_abuild(prev, h2)
    obc = emit_obc(prev, a_h)
    emit_tail(prev, obc, state[prev]["oh"])

    # ---- store: transpose res_all [l_part, bk] on PE, contiguous DMA out ----
    res_ps = posm.tile([n_bk, 128], F32, tag="obc", name="res_ps")
    nc.tensor.transpose(res_ps, res_all, ident)
    res_T = small.tile([n_bk, 128], F32, tag="resT", name="res_T")
    nc.vector.tensor_copy(res_T, res_ps)
    nc.sync.dma_start(out=out[:, :], in_=res_T)


_NC_CACHE = {}


def _get_nc(n_bk):
    if n_bk not in _NC_CACHE:
        _NC_CACHE[n_bk] = build_kernel(n_bk)
    return _NC_CACHE[n_bk]


def run(inputs, trace=False):
    context = np.ascontiguousarray(np.asarray(inputs["context"], dtype=np.float32))
    xs = context.reshape(BK, 2, L, D)
    shared = {
        "w1": np.ascontiguousarray(np.asarray(inputs["W1"], dtype=np.float32)),
        "b1": np.ascontiguousarray(np.asarray(inputs["b1"], dtype=np.float32)),
        "w2": np.ascontiguousarray(np.asarray(inputs["W2"], dtype=np.float32)),
        "b2": np.ascontiguousarray(np.asarray(inputs["b2"], dtype=np.float32)),
        "w3": np.ascontiguousarray(np.asarray(inputs["W3"], dtype=np.float32)),
        "b3": np.ascontiguousarray(np.asarray(inputs["b3"], dtype=np.float32)),
    }
    in_maps = [
        {"x": np.ascontiguousarray(xs[c * BK_PER_CORE : (c + 1) * BK_PER_CORE]), **shared}
        for c in range(N_CORES)
    ]
    nc = _get_nc(BK_PER_CORE)
    res = run_bass_kernel_spmd(nc, in_maps, list(range(N_CORES)), trace=trace)
    outs = [m["out"] for m in res.results]
    full = np.concatenate(outs, axis=0).reshape(B, K, L).astype(np.float32)
    return full, res


def kernel(**inputs) -> np.ndarray:
    full, _ = run(inputs, trace=False)
    return full
